# revision 15
# baseline (speedup 1.0000x reference)
"""Trainium2 Bass kernel for nn_MultiHeadAttention (B=2, S=2048, E=1024, H=16).

Sharding: 32 (batch, head) pairs across 8 cores -> core c owns batch c//4 and
heads 4*(c%4) .. 4*(c%4)+4. Each core computes its 4 heads' attention
(returning the full softmax matrix) plus a partial output projection; the host
sums the 4 partial projections per batch and adds b_out.

Device pipeline per core (all matmuls in float32r, full-rate at N>=512):
  1. qkT = (Wqk x^T + b)     [512, S]  (features on partitions, head-packed)
     V   = (x Wv^T + b)      [S, 256]  (seq on partitions)
  2. per head, per 128-row q block: scores psum -> Exp (scale=1/8 fused,
     accum_out = row sums) -> reciprocal -> normalize -> DMA attn out.
  3. per head: scores^T recomputed ([k,q] orientation), Exp -> expT,
     P@V accumulated over k into vals^T [64, q]; normalized by a
     ones-matmul broadcast of 1/rowsum and written to valsT sbuf.
  4. out_partial = vals^T.T @ w_outT accumulated over both head pairs.

This container's walrus accepts at most ONE sync wait per instruction; BIR is
post-processed to hoist extra waits onto EventSemaphore carriers, and the
TileContext exit drain is split the same way.
"""

import json
import sys

sys.path.insert(0, "/opt/trn_rl_repo")

import numpy as np  # noqa: E402

import concourse.bass as bass  # noqa: E402
import concourse.mybir as mybir  # noqa: E402
import concourse.tile as tile  # noqa: E402

F32 = mybir.dt.float32
F32R = mybir.dt.float32r
AF = mybir.ActivationFunctionType
MUL = mybir.AluOpType.mult
ADD = mybir.AluOpType.add

P = 128
B, S, E = 2, 2048, 1024
H, HD = 16, 64
NCORES = 8
HPC = 4  # heads per core
ESH = HPC * HD  # 256: this core's slice of the embedding
ESUB = E // P  # 8

# ---------------------------------------------------------------------------
# toolchain workarounds


def _patch_tile_drain():
    """Split the TileContext exit drain's waits (one per active proc) into
    one-wait carrier drains; this walrus rejects multi-wait instructions."""
    from concourse.vector_clock import ScopedClock, VectorClock

    def _drain_and_barrier(self, tick_clock, wait_clock):
        nc = self.nc
        gc = tick_clock.global_clock
        n = len(gc)
        for p in [i for i in range(n) if gc[i] > 0]:
            vec = [0] * n
            vec[p] = gc[p]
            carrier = nc.sync.drain()
            wait_clock.add_sem_waits(carrier.ins, ScopedClock({None: VectorClock(vec)}))
        nc.sync.drain()
        nc.all_engine_barrier()
        assert self.sems is not None
        popped = nc._tile_sem_poison_stack.pop()
        assert popped is self._sem_poison
        nc.clear_and_free_semaphores(list(self.sems.allocated().values()))
        nc.all_engine_barrier()

    tile.TileContext._drain_and_barrier = _drain_and_barrier


_KNOWN_ENGINES = {"PE", "DVE", "Activation", "Pool", "SP"}


def _legalize_waits(doc):
    """Hoist all but one on_wait per instruction onto EventSemaphore carriers."""
    for fn in doc.get("functions", []):
        for bb in fn.get("basic_blocks", fn.get("blocks", [])):
            out = []
            for inst in bb.get("instructions", []):
                si = inst.get("sync_info") or {}
                ow = si.get("on_wait") or []
                if len(ow) > 1:
                    eng = inst.get("engine")
                    assert eng in _KNOWN_ENGINES, (
                        f"multi-wait on unknown engine {eng}: {inst.get('name')}"
                    )
                    for i, w in enumerate(ow[:-1]):
                        carrier = {
                            "name": f"{inst['name']}-w{i}",
                            "opcode": "EventSemaphore",
                            "engine": eng,
                            "ins": [],
                            "outs": [],
                            "sync_info": {"on_update": [], "on_wait": [w]},
                        }
                        if "debug" in inst:
                            carrier["debug"] = inst["debug"]
                        out.append(carrier)
                    si["on_wait"] = [ow[-1]]
                    inst["sync_info"] = si
                out.append(inst)
            bb["instructions"] = out
    return doc


_PATCHED = False


def _apply_patches():
    global _PATCHED
    if _PATCHED:
        return
    _patch_tile_drain()
    orig = bass.Bass.to_json_bytes

    def patched(self):
        doc = json.loads(orig(self))
        return json.dumps(_legalize_waits(doc)).encode()

    bass.Bass.to_json_bytes = patched
    _PATCHED = True


# ---------------------------------------------------------------------------
# device program (SPMD; identical on all cores, sharding via input data)


def build_program(s=S, debug_taps=False):
    """One core's program. `s` is the sequence length (parametrized so the
    simulator can run a reduced-size version)."""
    _apply_patches()
    nqb = s // P  # q blocks of 128
    nkh = s // 1024  # 1024-wide halves of the k/q axis
    assert s % 1024 == 0

    nc = bass.Bass()
    xT = nc.declare_dram_parameter("xT", [E, s], F32, isOutput=False)
    wqkT = nc.declare_dram_parameter("wqkT", [E, 2 * ESH], F32, isOutput=False)
    wvT = nc.declare_dram_parameter("wvT", [E, ESH], F32, isOutput=False)
    bqk = nc.declare_dram_parameter("bqk", [2 * ESH], F32, isOutput=False)
    bv = nc.declare_dram_parameter("bv", [ESH], F32, isOutput=False)
    woutT = nc.declare_dram_parameter("woutT", [ESH, E], F32, isOutput=False)
    ones_in = nc.declare_dram_parameter("ones_in", [512], F32, isOutput=False)
    attn_o = nc.declare_dram_parameter("attn_o", [HPC, s, s], F32, isOutput=True)
    out_o = nc.declare_dram_parameter("out_o", [s, E], F32, isOutput=True)

    rs_bounce = nc.dram_tensor("rs_bounce", [HPC, P, nqb], F32)
    taps = {}
    if debug_taps:
        for nm, shape in [
            ("dbg_qkT", [P, 4, s]),
            ("dbg_v", [P, s // P, ESH]),
            ("dbg_valsT", [P, 2, s]),
            ("dbg_recipT", [HPC, s]),
            ("dbg_recip", [HPC, P, s // P]),
        ]:
            taps[nm] = nc.declare_dram_parameter(nm, shape, F32, isOutput=True)

    with tile.TileContext(nc) as tc:
        with (
            tc.tile_pool(name="persist", bufs=1) as pp,
            tc.tile_pool(name="ps_sc", bufs=2, space="PSUM") as ps_sc,
            tc.tile_pool(name="ps_vals", bufs=2, space="PSUM") as ps_vals,
            tc.tile_pool(name="ps_bc", bufs=1, space="PSUM") as ps_bc,
        ):
            # ---- constants (float32r via casting gpsimd DMAs)
            ones_sb = pp.tile([1, 512], F32R, tag="ones")
            nc.gpsimd.dma_start(ones_sb[:], ones_in[None, :])
            bqk_sb = pp.tile([1, 2 * ESH], F32R, tag="bqk")
            nc.gpsimd.dma_start(bqk_sb[:], bqk[None, :])
            bv_sb = pp.tile([1, ESH], F32R, tag="bv")
            nc.gpsimd.dma_start(bv_sb[:], bv[None, :])
            woutT_sb = pp.tile([P, 2, E], F32R, tag="woutT")
            nc.gpsimd.dma_start(
                woutT_sb[:], woutT.rearrange("(pr p) e -> p pr e", p=P)
            )

            # ---- persistent intermediates
            # qkT blocks: 0,1 = q heads (pair0, pair1), 2,3 = k heads
            qkT_sb = pp.tile([P, 4, s], F32R, tag="qkT")
            v_sb = pp.tile([P, nqb, ESH], F32R, tag="V")
            valsT_sb = pp.tile([P, 2, s], F32R, tag="valsT")
            recipT_sb = [
                pp.tile([1, s], F32R, tag=f"recipT{h}", name=f"recipT{h}")
                for h in range(HPC)
            ]
            rs_sb = [
                pp.tile([P, nkh, nqb], F32, tag=f"rs{h}", name=f"rs{h}")
                for h in range(HPC)
            ]
            recip_sb = [
                pp.tile([P, nqb], F32, tag=f"recip{h}", name=f"recip{h}")
                for h in range(HPC)
            ]

            # ---- phase 1: QKV projections, one 1024-wide s-half at a time
            with tc.tile_pool(name="xw", bufs=1) as xw:
                wqkT_sb = xw.tile([P, ESUB, 2 * ESH], F32R, tag="wqkT")
                nc.gpsimd.dma_start(
                    wqkT_sb[:], wqkT.rearrange("(es p) f -> p es f", p=P)
                )
                wvT_sb = xw.tile([P, ESUB, ESH], F32R, tag="wvT")
                nc.gpsimd.dma_start(
                    wvT_sb[:], wvT.rearrange("(es p) f -> p es f", p=P)
                )
                xv = xT.rearrange("(es p) s -> p es s", p=P)
                for sh in range(nkh):
                    xT_h = xw.tile([P, ESUB, 1024], F32R, tag="xTh", name=f"xT{sh}")
                    for es in range(ESUB):
                        nc.gpsimd.dma_start(
                            xT_h[:, es, :], xv[:, es, sh * 1024 : (sh + 1) * 1024]
                        )
                    # qkT = Wqk x^T + b  (4 feature blocks of 128)
                    for blk in range(4):
                        for sc in range(2):
                            s0 = sh * 1024 + sc * 512
                            pt = ps_vals.tile(
                                [P, 512], F32, tag="vals", name=f"qk{sh}_{blk}_{sc}"
                            )
                            for es in range(ESUB):
                                nc.tensor.matmul(
                                    pt[:],
                                    wqkT_sb[:, es, blk * P : (blk + 1) * P],
                                    xT_h[:, es, sc * 512 : (sc + 1) * 512],
                                    start=(es == 0),
                                    stop=False,
                                )
                            nc.tensor.matmul(
                                pt[:],
                                bqk_sb[:, blk * P : (blk + 1) * P],
                                ones_sb[:],
                                start=False,
                                stop=True,
                            )
                            nc.scalar.copy(
                                qkT_sb[:, blk, s0 : s0 + 512], pt[:]
                            )
                    # V = x Wv^T + b  (seq blocks of 128 on partitions)
                    for sb in range(8):
                        pt = ps_vals.tile(
                            [P, 512], F32, tag="vals", name=f"v{sh}_{sb}"
                        )
                        for es in range(ESUB):
                            nc.tensor.matmul(
                                pt[:, :ESH],
                                xT_h[:, es, sb * P : (sb + 1) * P],
                                wvT_sb[:, es, :],
                                start=(es == 0),
                                stop=False,
                            )
                        nc.tensor.matmul(
                            pt[:, :ESH],
                            ones_sb[:, 0:P],
                            bv_sb[:],
                            start=False,
                            stop=True,
                        )
                        nc.vector.tensor_copy(
                            v_sb[:, sh * 8 + sb, :], pt[:, :ESH]
                        )

            # ---- phases 2+3, pair by pair
            _wp_cm = tc.tile_pool(name="work", bufs=3)
            wp = _wp_cm.__enter__()

            def phase2(h):
                hp = 64 * (h % 2)
                qblk = h // 2
                kblk = 2 + h // 2
                for qb in range(nqb):
                    at = wp.tile([P, s], F32, tag="attn", name=f"attn{h}_{qb}")
                    for kh in range(nkh):
                        pt = ps_sc.tile([P, 1024], F32, tag="sc", name=f"s{h}_{qb}_{kh}")
                        for i in range(2):
                            kc = kh * 1024 + i * 512
                            nc.tensor.matmul(
                                pt[:, i * 512 : (i + 1) * 512],
                                qkT_sb[hp : hp + 64, qblk, qb * P : (qb + 1) * P],
                                qkT_sb[hp : hp + 64, kblk, kc : kc + 512],
                                start=True,
                                stop=True,
                            )
                        nc.scalar.activation(
                            at[:, kh * 1024 : (kh + 1) * 1024],
                            pt[:],
                            AF.Exp,
                            scale=0.125,
                            accum_out=rs_sb[h][:, kh, qb : qb + 1],
                        )
                    # rowsum = sum of the nkh partial sums -> reciprocal
                    if nkh == 2:
                        nc.vector.tensor_tensor(
                            recip_sb[h][:, qb : qb + 1],
                            rs_sb[h][:, 0, qb : qb + 1],
                            rs_sb[h][:, 1, qb : qb + 1],
                            ADD,
                        )
                        nc.vector.reciprocal(
                            recip_sb[h][:, qb : qb + 1], recip_sb[h][:, qb : qb + 1]
                        )
                    else:
                        nc.vector.reciprocal(
                            recip_sb[h][:, qb : qb + 1], rs_sb[h][:, 0, qb : qb + 1]
                        )
                    nc.vector.tensor_scalar_mul(
                        at[:], at[:], recip_sb[h][:, qb : qb + 1]
                    )
                    nc.sync.dma_start(attn_o[h, qb * P : (qb + 1) * P, :], at[:])
                # bounce recip [P, nqb] -> recipT [1, s] (free-major), f32r
                nc.sync.dma_start(rs_bounce[h], recip_sb[h][:])
                nc.gpsimd.dma_start(
                    recipT_sb[h].rearrange("o (q p) -> o q p", p=P),
                    rs_bounce[h].rearrange("p q -> q p")[None],
                )

            def phase3(h):
                hp = 64 * (h % 2)
                qblk = h // 2
                kblk = 2 + h // 2
                pair = h // 2
                for qh in range(nkh):  # 1024-wide q halves
                    vts = [
                        ps_vals.tile([P, 512], F32, tag="vals", name=f"vt{h}_{qh}_0"),
                        ps_vals.tile([P, 512], F32, tag="vals", name=f"vt{h}_{qh}_1"),
                    ]
                    for kb in range(nqb):
                        pt = ps_sc.tile([P, 1024], F32, tag="sc", name=f"t{h}_{qh}_{kb}")
                        for i in range(2):
                            qc = qh * 1024 + i * 512
                            nc.tensor.matmul(
                                pt[:, i * 512 : (i + 1) * 512],
                                qkT_sb[hp : hp + 64, kblk, kb * P : (kb + 1) * P],
                                qkT_sb[hp : hp + 64, qblk, qc : qc + 512],
                                start=True,
                                stop=True,
                            )
                        et = wp.tile([P, 1024], F32R, tag="expT", name=f"e{h}_{qh}_{kb}")
                        nc.scalar.activation(et[:], pt[:], AF.Exp, scale=0.125)
                        for i in range(2):
                            nc.tensor.matmul(
                                vts[i][0:64, :],
                                v_sb[:, kb, h * 64 : (h + 1) * 64],
                                et[:, i * 512 : (i + 1) * 512],
                                start=(kb == 0),
                                stop=(kb == nqb - 1),
                                skip_group_check=True,
                            )
                    # broadcast 1/rowsum across the 64 d-partitions via ones-matmul
                    bc = ps_bc.tile([P, 1024], F32, tag="bcast", name=f"bc{h}_{qh}")
                    for i in range(2):
                        qc = qh * 1024 + i * 512
                        nc.tensor.matmul(
                            bc[0:64, i * 512 : (i + 1) * 512],
                            ones_sb[:, 0:64],
                            recipT_sb[h][:, qc : qc + 512],
                            start=True,
                            stop=True,
                        )
                    bcs = wp.tile([P, 1024], F32, tag="bc_sb", name=f"bcs{h}_{qh}")
                    nc.scalar.copy(bcs[0:64, :], bc[0:64, :])
                    for i in range(2):
                        qc = qh * 1024 + i * 512
                        nc.vector.tensor_tensor(
                            valsT_sb[hp : hp + 64, pair, qc : qc + 512],
                            vts[i][0:64, :],
                            bcs[0:64, i * 512 : (i + 1) * 512],
                            MUL,
                        )

            for pair in range(2):
                for h in (2 * pair, 2 * pair + 1):
                    phase2(h)
                for h in (2 * pair, 2 * pair + 1):
                    phase3(h)

            # ---- phase 4: out_partial = valsT.T @ woutT (accumulate pairs)
            for sb in range(nqb):
                pt = ps_sc.tile([P, 1024], F32, tag="sc", name=f"o{sb}")
                for fc in range(2):
                    for pair in range(2):
                        nc.tensor.matmul(
                            pt[:, fc * 512 : (fc + 1) * 512],
                            valsT_sb[:, pair, sb * P : (sb + 1) * P],
                            woutT_sb[:, pair, fc * 512 : (fc + 1) * 512],
                            start=(pair == 0),
                            stop=(pair == 1),
                        )
                ot = wp.tile([P, 1024], F32, tag="out", name=f"ot{sb}")
                nc.scalar.copy(ot[:], pt[:])
                nc.sync.dma_start(out_o[sb * P : (sb + 1) * P, :], ot[:])

            if debug_taps:
                nc.sync.dma_start(taps["dbg_qkT"][:], qkT_sb[:].bitcast(F32))
                nc.sync.dma_start(taps["dbg_v"][:], v_sb[:].bitcast(F32))
                nc.sync.dma_start(taps["dbg_valsT"][:], valsT_sb[:].bitcast(F32))
                for h in range(HPC):
                    nc.sync.dma_start(
                        taps["dbg_recipT"][h][None, :], recipT_sb[h].bitcast(F32)
                    )
                    nc.sync.dma_start(taps["dbg_recip"][h], recip_sb[h][:])

            _wp_cm.__exit__(None, None, None)

    return nc


# ---------------------------------------------------------------------------
# host side: shard, run, gather

_CACHED = None


def _get_runner():
    """Build the program and a cached sharded executable (compile once)."""
    global _CACHED
    if _CACHED is not None:
        return _CACHED

    import jax
    from jax.sharding import Mesh, PartitionSpec
    from jax.experimental.shard_map import shard_map

    from concourse import bass2jax
    from concourse.bass2jax import _bass_exec_p, install_neuronx_cc_hook

    nc = build_program()
    install_neuronx_cc_hook()

    partition_name = (
        nc.partition_id_tensor.name if nc.partition_id_tensor else None
    )
    in_names = []
    out_names = []
    out_avals = []
    out_shapes = []
    for alloc in nc.m.functions[0].allocations:
        if not isinstance(alloc, mybir.MemoryLocationSet):
            continue
        name = alloc.memorylocations[0].name
        if alloc.kind == "ExternalInput":
            if name != partition_name:
                in_names.append(name)
        elif alloc.kind == "ExternalOutput":
            shape = tuple(alloc.tensor_shape)
            dtype = mybir.dt.np(alloc.dtype)
            out_names.append(name)
            out_avals.append(jax.core.ShapedArray(shape, dtype))
            out_shapes.append((shape, dtype))
    n_params = len(in_names)
    all_names = in_names + out_names
    if partition_name is not None:
        all_names = all_names + [partition_name]

    def _body(*args):
        operands = list(args)
        if partition_name is not None:
            operands.append(bass2jax.partition_id_tensor())
        outs = _bass_exec_p.bind(
            *operands,
            out_avals=tuple(out_avals),
            in_names=tuple(all_names),
            out_names=tuple(out_names),
            lowering_input_output_aliases=(),
            sim_require_finite=True,
            sim_require_nnan=True,
            nc=nc,
        )
        return tuple(outs)

    devices = jax.devices()[:NCORES]
    mesh = Mesh(np.asarray(devices), ("core",))
    specs = (PartitionSpec("core"),) * (n_params + len(out_names))
    sharded = jax.jit(
        shard_map(
            _body,
            mesh=mesh,
            in_specs=specs,
            out_specs=(PartitionSpec("core"),) * len(out_names),
            check_rep=False,
        ),
        keep_unused=True,
    )
    _CACHED = (sharded, in_names, out_names, out_shapes, mesh)
    return _CACHED


def _shard_inputs(x, w_qkv, b_qkv, w_out):
    """Per-core input dicts (all fp32, contiguous)."""
    ones = np.ones((512,), dtype=np.float32)
    per_core = []
    for c in range(NCORES):
        b = c // 4
        h0 = (c % 4) * HPC
        # reference packs qkv per head: w_qkv row f -> head f//(3*HD),
        # q/k/v at offsets 0/HD/2*HD within each 3*HD group
        q_rows = np.concatenate(
            [3 * HD * h + np.arange(HD) for h in range(h0, h0 + HPC)]
        )
        k_rows = q_rows + HD
        v_rows = q_rows + 2 * HD
        qk_rows = np.concatenate([q_rows, k_rows])
        e_cols = np.arange(h0 * HD, h0 * HD + ESH)
        per_core.append(
            {
                "xT": np.ascontiguousarray(x[b].T),
                "wqkT": np.ascontiguousarray(w_qkv[qk_rows].T),
                "wvT": np.ascontiguousarray(w_qkv[v_rows].T),
                "bqk": np.ascontiguousarray(b_qkv[qk_rows]),
                "bv": np.ascontiguousarray(b_qkv[v_rows]),
                "woutT": np.ascontiguousarray(w_out[:, e_cols].T),
                "ones_in": ones,
            }
        )
    return per_core


def run_cores(x, w_qkv, b_qkv, w_out):
    """Run the SPMD program; returns per-core {attn_o, out_o} numpy arrays."""
    sharded, in_names, out_names, out_shapes, mesh = _get_runner()
    per_core = _shard_inputs(x, w_qkv, b_qkv, w_out)
    concat_in = [
        np.concatenate([per_core[c][n] for c in range(NCORES)], axis=0)
        for n in in_names
    ]
    concat_zeros = [
        np.zeros((NCORES * sh[0], *sh[1:]), dt) for (sh, dt) in out_shapes
    ]
    out_arrs = sharded(*concat_in, *concat_zeros)
    return [
        {
            n: np.asarray(out_arrs[i]).reshape(NCORES, *out_shapes[i][0])[c]
            for i, n in enumerate(out_names)
        }
        for c in range(NCORES)
    ]


def kernel(x, w_qkv, b_qkv, w_out, b_out):
    x = np.asarray(x, dtype=np.float32)
    w_qkv = np.asarray(w_qkv, dtype=np.float32)
    b_qkv = np.asarray(b_qkv, dtype=np.float32)
    w_out = np.asarray(w_out, dtype=np.float32)
    b_out = np.asarray(b_out, dtype=np.float32)

    results = run_cores(x, w_qkv, b_qkv, w_out)

    attn = np.empty((B, H, S, S), dtype=np.float32)
    out = np.zeros((B, S, E), dtype=np.float32)
    for c in range(NCORES):
        b = c // 4
        h0 = (c % 4) * HPC
        attn[b, h0 : h0 + HPC] = results[c]["attn_o"]
        out[b] += results[c]["out_o"]
    out += b_out
    return out, attn


# revision 16
# speedup vs baseline: 16.8090x; 16.8090x over previous
"""Trainium2 Bass kernel for nn_MultiHeadAttention (B=2, S=2048, E=1024, H=16).

Sharding: 32 (batch, head) pairs across 8 cores -> core c owns batch c//4 and
heads 4*(c%4) .. 4*(c%4)+4. Each core computes its 4 heads' attention
(returning the full softmax matrix) plus a partial output projection; the host
sums the 4 partial projections per batch and adds b_out.

Device pipeline per core (all matmuls in float32r, full-rate at N>=512):
  1. qkT = (Wqk x^T + b)     [512, S]  (features on partitions, head-packed)
     V   = (x Wv^T + b)      [S, 256]  (seq on partitions)
  2. per head, per 128-row q block: scores psum -> Exp (scale=1/8 fused,
     accum_out = row sums) -> reciprocal -> normalize -> DMA attn out.
  3. per head: scores^T recomputed ([k,q] orientation), Exp -> expT,
     P@V accumulated over k into vals^T [64, q]; normalized by a
     ones-matmul broadcast of 1/rowsum and written to valsT sbuf.
  4. out_partial = vals^T.T @ w_outT accumulated over both head pairs.

This container's walrus accepts at most ONE sync wait per instruction; BIR is
post-processed to hoist extra waits onto EventSemaphore carriers, and the
TileContext exit drain is split the same way.
"""

import json
import sys

sys.path.insert(0, "/opt/trn_rl_repo")

import numpy as np  # noqa: E402

import concourse.bass as bass  # noqa: E402
import concourse.mybir as mybir  # noqa: E402
import concourse.tile as tile  # noqa: E402

F32 = mybir.dt.float32
F32R = mybir.dt.float32r
AF = mybir.ActivationFunctionType
MUL = mybir.AluOpType.mult
ADD = mybir.AluOpType.add

P = 128
B, S, E = 2, 2048, 1024
H, HD = 16, 64
NCORES = 8
HPC = 4  # heads per core
ESH = HPC * HD  # 256: this core's slice of the embedding
ESUB = E // P  # 8

# ---------------------------------------------------------------------------
# toolchain workarounds


def _patch_tile_drain():
    """Split the TileContext exit drain's waits (one per active proc) into
    one-wait carrier drains; this walrus rejects multi-wait instructions."""
    from concourse.vector_clock import ScopedClock, VectorClock

    def _drain_and_barrier(self, tick_clock, wait_clock):
        nc = self.nc
        gc = tick_clock.global_clock
        n = len(gc)
        for p in [i for i in range(n) if gc[i] > 0]:
            vec = [0] * n
            vec[p] = gc[p]
            carrier = nc.sync.drain()
            wait_clock.add_sem_waits(carrier.ins, ScopedClock({None: VectorClock(vec)}))
        nc.sync.drain()
        nc.all_engine_barrier()
        assert self.sems is not None
        popped = nc._tile_sem_poison_stack.pop()
        assert popped is self._sem_poison
        nc.clear_and_free_semaphores(list(self.sems.allocated().values()))
        nc.all_engine_barrier()

    tile.TileContext._drain_and_barrier = _drain_and_barrier


_KNOWN_ENGINES = {"PE", "DVE", "Activation", "Pool", "SP"}


def _legalize_waits(doc):
    """Hoist all but one on_wait per instruction onto EventSemaphore carriers."""
    for fn in doc.get("functions", []):
        for bb in fn.get("basic_blocks", fn.get("blocks", [])):
            out = []
            for inst in bb.get("instructions", []):
                si = inst.get("sync_info") or {}
                ow = si.get("on_wait") or []
                if len(ow) > 1:
                    eng = inst.get("engine")
                    assert eng in _KNOWN_ENGINES, (
                        f"multi-wait on unknown engine {eng}: {inst.get('name')}"
                    )
                    for i, w in enumerate(ow[:-1]):
                        carrier = {
                            "name": f"{inst['name']}-w{i}",
                            "opcode": "EventSemaphore",
                            "engine": eng,
                            "ins": [],
                            "outs": [],
                            "sync_info": {"on_update": [], "on_wait": [w]},
                        }
                        if "debug" in inst:
                            carrier["debug"] = inst["debug"]
                        out.append(carrier)
                    si["on_wait"] = [ow[-1]]
                    inst["sync_info"] = si
                out.append(inst)
            bb["instructions"] = out
    return doc


_PATCHED = False


def _apply_patches():
    global _PATCHED
    if _PATCHED:
        return
    _patch_tile_drain()
    orig = bass.Bass.to_json_bytes

    def patched(self):
        doc = json.loads(orig(self))
        return json.dumps(_legalize_waits(doc)).encode()

    bass.Bass.to_json_bytes = patched
    _PATCHED = True


# ---------------------------------------------------------------------------
# device program (SPMD; identical on all cores, sharding via input data)


def build_program(s=S, debug_taps=False):
    """One core's program. `s` is the sequence length (parametrized so the
    simulator can run a reduced-size version)."""
    _apply_patches()
    nqb = s // P  # q blocks of 128
    nkh = s // 1024  # 1024-wide halves of the k/q axis
    assert s % 1024 == 0

    nc = bass.Bass()
    xT = nc.declare_dram_parameter("xT", [E, s], F32, isOutput=False)
    wqkT = nc.declare_dram_parameter("wqkT", [E, 2 * ESH], F32, isOutput=False)
    wvT = nc.declare_dram_parameter("wvT", [E, ESH], F32, isOutput=False)
    bqk = nc.declare_dram_parameter("bqk", [2 * ESH], F32, isOutput=False)
    bv = nc.declare_dram_parameter("bv", [ESH], F32, isOutput=False)
    woutT = nc.declare_dram_parameter("woutT", [ESH, E], F32, isOutput=False)
    ones_in = nc.declare_dram_parameter("ones_in", [512], F32, isOutput=False)
    attn_o = nc.declare_dram_parameter("attn_o", [HPC, s, s], F32, isOutput=True)
    out_o = nc.declare_dram_parameter("out_o", [s, E], F32, isOutput=True)

    rs_bounce = nc.dram_tensor("rs_bounce", [HPC, P, nqb], F32)
    taps = {}
    if debug_taps:
        for nm, shape in [
            ("dbg_qkT", [P, 4, s]),
            ("dbg_v", [P, s // P, ESH]),
            ("dbg_valsT", [P, 2, s]),
            ("dbg_recipT", [HPC, s]),
            ("dbg_recip", [HPC, P, s // P]),
        ]:
            taps[nm] = nc.declare_dram_parameter(nm, shape, F32, isOutput=True)

    with tile.TileContext(nc) as tc:
        with (
            tc.tile_pool(name="persist", bufs=1) as pp,
            tc.tile_pool(name="ps_sc", bufs=2, space="PSUM") as ps_sc,
            tc.tile_pool(name="ps_vals", bufs=2, space="PSUM") as ps_vals,
            tc.tile_pool(name="ps_bc", bufs=1, space="PSUM") as ps_bc,
        ):
            # ---- constants (float32r via casting gpsimd DMAs)
            ones_sb = pp.tile([1, 512], F32R, tag="ones")
            nc.gpsimd.dma_start(ones_sb[:], ones_in[None, :])
            bqk_sb = pp.tile([1, 2 * ESH], F32R, tag="bqk")
            nc.gpsimd.dma_start(bqk_sb[:], bqk[None, :])
            bv_sb = pp.tile([1, ESH], F32R, tag="bv")
            nc.gpsimd.dma_start(bv_sb[:], bv[None, :])
            woutT_sb = pp.tile([P, 2, E], F32R, tag="woutT")
            nc.gpsimd.dma_start(
                woutT_sb[:], woutT.rearrange("(pr p) e -> p pr e", p=P)
            )

            # ---- persistent intermediates
            # qkT blocks: 0,1 = q heads (pair0, pair1), 2,3 = k heads
            qkT_sb = pp.tile([P, 4, s], F32R, tag="qkT")
            v_sb = pp.tile([P, nqb, ESH], F32R, tag="V")
            valsT_sb = pp.tile([P, 2, s], F32R, tag="valsT")
            recipT_sb = [
                pp.tile([1, s], F32R, tag=f"recipT{h}", name=f"recipT{h}")
                for h in range(HPC)
            ]
            rs_sb = [
                pp.tile([P, nkh, nqb], F32, tag=f"rs{h}", name=f"rs{h}")
                for h in range(HPC)
            ]
            recip_sb = [
                pp.tile([P, nqb], F32, tag=f"recip{h}", name=f"recip{h}")
                for h in range(HPC)
            ]

            # ---- phase 1: QKV projections, one 1024-wide s-half at a time
            with tc.tile_pool(name="xw", bufs=1) as xw:
                wqkT_sb = xw.tile([P, ESUB, 2 * ESH], F32R, tag="wqkT")
                nc.gpsimd.dma_start(
                    wqkT_sb[:], wqkT.rearrange("(es p) f -> p es f", p=P)
                )
                wvT_sb = xw.tile([P, ESUB, ESH], F32R, tag="wvT")
                nc.gpsimd.dma_start(
                    wvT_sb[:], wvT.rearrange("(es p) f -> p es f", p=P)
                )
                xv = xT.rearrange("(es p) s -> p es s", p=P)
                for sh in range(nkh):
                    xT_h = xw.tile([P, ESUB, 1024], F32R, tag="xTh", name=f"xT{sh}")
                    for es in range(ESUB):
                        nc.gpsimd.dma_start(
                            xT_h[:, es, :], xv[:, es, sh * 1024 : (sh + 1) * 1024]
                        )
                    # qkT = Wqk x^T + b  (4 feature blocks of 128)
                    for blk in range(4):
                        for sc in range(2):
                            s0 = sh * 1024 + sc * 512
                            pt = ps_vals.tile(
                                [P, 512], F32, tag="vals", name=f"qk{sh}_{blk}_{sc}"
                            )
                            for es in range(ESUB):
                                nc.tensor.matmul(
                                    pt[:],
                                    wqkT_sb[:, es, blk * P : (blk + 1) * P],
                                    xT_h[:, es, sc * 512 : (sc + 1) * 512],
                                    start=(es == 0),
                                    stop=False,
                                )
                            nc.tensor.matmul(
                                pt[:],
                                bqk_sb[:, blk * P : (blk + 1) * P],
                                ones_sb[:],
                                start=False,
                                stop=True,
                            )
                            nc.scalar.copy(
                                qkT_sb[:, blk, s0 : s0 + 512], pt[:]
                            )
                    # V = x Wv^T + b  (seq blocks of 128 on partitions)
                    for sb in range(8):
                        pt = ps_vals.tile(
                            [P, 512], F32, tag="vals", name=f"v{sh}_{sb}"
                        )
                        for es in range(ESUB):
                            nc.tensor.matmul(
                                pt[:, :ESH],
                                xT_h[:, es, sb * P : (sb + 1) * P],
                                wvT_sb[:, es, :],
                                start=(es == 0),
                                stop=False,
                            )
                        nc.tensor.matmul(
                            pt[:, :ESH],
                            ones_sb[:, 0:P],
                            bv_sb[:],
                            start=False,
                            stop=True,
                        )
                        nc.vector.tensor_copy(
                            v_sb[:, sh * 8 + sb, :], pt[:, :ESH]
                        )

            # ---- phases 2+3, pair by pair
            _wp_cm = tc.tile_pool(name="work", bufs=3)
            wp = _wp_cm.__enter__()

            def phase2(h):
                hp = 64 * (h % 2)
                qblk = h // 2
                kblk = 2 + h // 2
                for qb in range(nqb):
                    at = wp.tile([P, s], F32, tag="attn", name=f"attn{h}_{qb}")
                    for kh in range(nkh):
                        pt = ps_sc.tile([P, 1024], F32, tag="sc", name=f"s{h}_{qb}_{kh}")
                        for i in range(2):
                            kc = kh * 1024 + i * 512
                            nc.tensor.matmul(
                                pt[:, i * 512 : (i + 1) * 512],
                                qkT_sb[hp : hp + 64, qblk, qb * P : (qb + 1) * P],
                                qkT_sb[hp : hp + 64, kblk, kc : kc + 512],
                                start=True,
                                stop=True,
                            )
                        nc.scalar.activation(
                            at[:, kh * 1024 : (kh + 1) * 1024],
                            pt[:],
                            AF.Exp,
                            scale=0.125,
                            accum_out=rs_sb[h][:, kh, qb : qb + 1],
                        )
                    # rowsum = sum of the nkh partial sums -> reciprocal
                    if nkh == 2:
                        nc.vector.tensor_tensor(
                            recip_sb[h][:, qb : qb + 1],
                            rs_sb[h][:, 0, qb : qb + 1],
                            rs_sb[h][:, 1, qb : qb + 1],
                            ADD,
                        )
                        nc.vector.reciprocal(
                            recip_sb[h][:, qb : qb + 1], recip_sb[h][:, qb : qb + 1]
                        )
                    else:
                        nc.vector.reciprocal(
                            recip_sb[h][:, qb : qb + 1], rs_sb[h][:, 0, qb : qb + 1]
                        )
                    nc.vector.tensor_scalar_mul(
                        at[:], at[:], recip_sb[h][:, qb : qb + 1]
                    )
                    nc.sync.dma_start(attn_o[h, qb * P : (qb + 1) * P, :], at[:])
                # bounce recip [P, nqb] -> recipT [1, s] (free-major), f32r
                nc.sync.dma_start(rs_bounce[h], recip_sb[h][:])
                nc.gpsimd.dma_start(
                    recipT_sb[h].rearrange("o (q p) -> o q p", p=P),
                    rs_bounce[h].rearrange("p q -> q p")[None],
                )

            def phase3(h):
                hp = 64 * (h % 2)
                qblk = h // 2
                kblk = 2 + h // 2
                pair = h // 2
                for qh in range(nkh):  # 1024-wide q halves
                    vts = [
                        ps_vals.tile([P, 512], F32, tag="vals", name=f"vt{h}_{qh}_0"),
                        ps_vals.tile([P, 512], F32, tag="vals", name=f"vt{h}_{qh}_1"),
                    ]
                    for kb in range(nqb):
                        pt = ps_sc.tile([P, 1024], F32, tag="sc", name=f"t{h}_{qh}_{kb}")
                        for i in range(2):
                            qc = qh * 1024 + i * 512
                            nc.tensor.matmul(
                                pt[:, i * 512 : (i + 1) * 512],
                                qkT_sb[hp : hp + 64, kblk, kb * P : (kb + 1) * P],
                                qkT_sb[hp : hp + 64, qblk, qc : qc + 512],
                                start=True,
                                stop=True,
                            )
                        et = wp.tile([P, 1024], F32R, tag="expT", name=f"e{h}_{qh}_{kb}")
                        nc.scalar.activation(et[:], pt[:], AF.Exp, scale=0.125)
                        for i in range(2):
                            nc.tensor.matmul(
                                vts[i][0:64, :],
                                v_sb[:, kb, h * 64 : (h + 1) * 64],
                                et[:, i * 512 : (i + 1) * 512],
                                start=(kb == 0),
                                stop=(kb == nqb - 1),
                                skip_group_check=True,
                            )
                    # broadcast 1/rowsum across the 64 d-partitions via ones-matmul
                    bc = ps_bc.tile([P, 1024], F32, tag="bcast", name=f"bc{h}_{qh}")
                    for i in range(2):
                        qc = qh * 1024 + i * 512
                        nc.tensor.matmul(
                            bc[0:64, i * 512 : (i + 1) * 512],
                            ones_sb[:, 0:64],
                            recipT_sb[h][:, qc : qc + 512],
                            start=True,
                            stop=True,
                        )
                    bcs = wp.tile([P, 1024], F32, tag="bc_sb", name=f"bcs{h}_{qh}")
                    nc.scalar.copy(bcs[0:64, :], bc[0:64, :])
                    for i in range(2):
                        qc = qh * 1024 + i * 512
                        nc.vector.tensor_tensor(
                            valsT_sb[hp : hp + 64, pair, qc : qc + 512],
                            vts[i][0:64, :],
                            bcs[0:64, i * 512 : (i + 1) * 512],
                            MUL,
                        )

            for pair in range(2):
                for h in (2 * pair, 2 * pair + 1):
                    phase2(h)
                for h in (2 * pair, 2 * pair + 1):
                    phase3(h)

            # ---- phase 4: out_partial = valsT.T @ woutT (accumulate pairs)
            for sb in range(nqb):
                pt = ps_sc.tile([P, 1024], F32, tag="sc", name=f"o{sb}")
                for fc in range(2):
                    for pair in range(2):
                        nc.tensor.matmul(
                            pt[:, fc * 512 : (fc + 1) * 512],
                            valsT_sb[:, pair, sb * P : (sb + 1) * P],
                            woutT_sb[:, pair, fc * 512 : (fc + 1) * 512],
                            start=(pair == 0),
                            stop=(pair == 1),
                        )
                ot = wp.tile([P, 1024], F32, tag="out", name=f"ot{sb}")
                nc.scalar.copy(ot[:], pt[:])
                nc.sync.dma_start(out_o[sb * P : (sb + 1) * P, :], ot[:])

            if debug_taps:
                nc.sync.dma_start(taps["dbg_qkT"][:], qkT_sb[:].bitcast(F32))
                nc.sync.dma_start(taps["dbg_v"][:], v_sb[:].bitcast(F32))
                nc.sync.dma_start(taps["dbg_valsT"][:], valsT_sb[:].bitcast(F32))
                for h in range(HPC):
                    nc.sync.dma_start(
                        taps["dbg_recipT"][h][None, :], recipT_sb[h].bitcast(F32)
                    )
                    nc.sync.dma_start(taps["dbg_recip"][h], recip_sb[h][:])

            _wp_cm.__exit__(None, None, None)

    return nc


# ---------------------------------------------------------------------------
# host side: shard, run, gather

_CACHED = {}
_NC = None


def _get_runner(loop_n=1):
    """Build the program and a cached sharded executable (compile once).

    loop_n > 1 chains that many kernel executions inside one jit (each
    iteration's outputs feed the next's output-buffer operands), for
    amortized device-time measurement.
    """
    global _NC
    if loop_n in _CACHED:
        return _CACHED[loop_n]

    import jax
    from jax.sharding import Mesh, PartitionSpec
    from jax.experimental.shard_map import shard_map

    from concourse import bass2jax
    from concourse.bass2jax import _bass_exec_p, install_neuronx_cc_hook

    if _NC is None:
        _NC = build_program()
    nc = _NC
    install_neuronx_cc_hook()

    partition_name = (
        nc.partition_id_tensor.name if nc.partition_id_tensor else None
    )
    in_names = []
    out_names = []
    out_avals = []
    out_shapes = []
    for alloc in nc.m.functions[0].allocations:
        if not isinstance(alloc, mybir.MemoryLocationSet):
            continue
        name = alloc.memorylocations[0].name
        if alloc.kind == "ExternalInput":
            if name != partition_name:
                in_names.append(name)
        elif alloc.kind == "ExternalOutput":
            shape = tuple(alloc.tensor_shape)
            dtype = mybir.dt.np(alloc.dtype)
            out_names.append(name)
            out_avals.append(jax.core.ShapedArray(shape, dtype))
            out_shapes.append((shape, dtype))
    n_params = len(in_names)
    all_names = in_names + out_names
    if partition_name is not None:
        all_names = all_names + [partition_name]

    def _body(*args):
        ins = list(args[:n_params])
        outs = list(args[n_params:])
        for _ in range(loop_n):
            operands = ins + outs
            if partition_name is not None:
                operands.append(bass2jax.partition_id_tensor())
            outs = list(
                _bass_exec_p.bind(
                    *operands,
                    out_avals=tuple(out_avals),
                    in_names=tuple(all_names),
                    out_names=tuple(out_names),
                    lowering_input_output_aliases=(),
                    sim_require_finite=True,
                    sim_require_nnan=True,
                    nc=nc,
                )
            )
        return tuple(outs)

    devices = jax.devices()[:NCORES]
    mesh = Mesh(np.asarray(devices), ("core",))
    specs = (PartitionSpec("core"),) * (n_params + len(out_names))
    sharded = jax.jit(
        shard_map(
            _body,
            mesh=mesh,
            in_specs=specs,
            out_specs=(PartitionSpec("core"),) * len(out_names),
            check_rep=False,
        ),
        keep_unused=True,
    )
    _CACHED[loop_n] = (sharded, in_names, out_names, out_shapes, mesh)
    return _CACHED[loop_n]


def _shard_inputs(x, w_qkv, b_qkv, w_out):
    """Per-core input dicts (all fp32, contiguous)."""
    ones = np.ones((512,), dtype=np.float32)
    per_core = []
    for c in range(NCORES):
        b = c // 4
        h0 = (c % 4) * HPC
        # reference packs qkv per head: w_qkv row f -> head f//(3*HD),
        # q/k/v at offsets 0/HD/2*HD within each 3*HD group
        q_rows = np.concatenate(
            [3 * HD * h + np.arange(HD) for h in range(h0, h0 + HPC)]
        )
        k_rows = q_rows + HD
        v_rows = q_rows + 2 * HD
        qk_rows = np.concatenate([q_rows, k_rows])
        e_cols = np.arange(h0 * HD, h0 * HD + ESH)
        per_core.append(
            {
                "xT": np.ascontiguousarray(x[b].T),
                "wqkT": np.ascontiguousarray(w_qkv[qk_rows].T),
                "wvT": np.ascontiguousarray(w_qkv[v_rows].T),
                "bqk": np.ascontiguousarray(b_qkv[qk_rows]),
                "bv": np.ascontiguousarray(b_qkv[v_rows]),
                "woutT": np.ascontiguousarray(w_out[:, e_cols].T),
                "ones_in": ones,
            }
        )
    return per_core


def run_cores(x, w_qkv, b_qkv, w_out):
    """Run the SPMD program; returns per-core {attn_o, out_o} numpy arrays."""
    sharded, in_names, out_names, out_shapes, mesh = _get_runner()
    per_core = _shard_inputs(x, w_qkv, b_qkv, w_out)
    concat_in = [
        np.concatenate([per_core[c][n] for c in range(NCORES)], axis=0)
        for n in in_names
    ]
    concat_zeros = [
        np.zeros((NCORES * sh[0], *sh[1:]), dt) for (sh, dt) in out_shapes
    ]
    out_arrs = sharded(*concat_in, *concat_zeros)
    return [
        {
            n: np.asarray(out_arrs[i]).reshape(NCORES, *out_shapes[i][0])[c]
            for i, n in enumerate(out_names)
        }
        for c in range(NCORES)
    ]


def kernel(x, w_qkv, b_qkv, w_out, b_out):
    x = np.asarray(x, dtype=np.float32)
    w_qkv = np.asarray(w_qkv, dtype=np.float32)
    b_qkv = np.asarray(b_qkv, dtype=np.float32)
    w_out = np.asarray(w_out, dtype=np.float32)
    b_out = np.asarray(b_out, dtype=np.float32)

    results = run_cores(x, w_qkv, b_qkv, w_out)

    attn = np.empty((B, H, S, S), dtype=np.float32)
    out = np.zeros((B, S, E), dtype=np.float32)
    for c in range(NCORES):
        b = c // 4
        h0 = (c % 4) * HPC
        attn[b, h0 : h0 + HPC] = results[c]["attn_o"]
        out[b] += results[c]["out_o"]
    out += b_out
    return out, attn
